# revision 16
# baseline (speedup 1.0000x reference)
"""InputScaledQuantLinear on 8 TRN2 NeuronCores.

out = dq(fp8_quant(x / s)) * s @ W^T + bias
    = s * (q @ W^T) + bias          (per-tensor scale s folded into W)

Sharding: x rows split 8 ways (data parallel), weight/bias replicated.
No cross-core communication; host concatenates the 8 output shards.
"""

import numpy as np
from contextlib import ExitStack

import concourse.bass as bass
import concourse.mybir as mybir
import concourse.tile as tile
from concourse import bacc
from concourse.bass_utils import run_bass_kernel_spmd

N_CORES = 8
N, IN, OUT = 32768, 2048, 2048
NS = N // N_CORES          # 4096 rows per core
N_CHUNK = 512              # token rows processed per outer iteration
K_TILES = IN // 128        # 16
O_BANKS = OUT // 512       # 4

_cache = {}


def build(scale: float):
    nc = bacc.Bacc(trn_type="TRN2")
    x = nc.dram_tensor("x", [NS, IN], mybir.dt.bfloat16, kind="ExternalInput")
    w = nc.dram_tensor("weight", [OUT, IN], mybir.dt.bfloat16, kind="ExternalInput")
    b = nc.dram_tensor("bias", [OUT], mybir.dt.bfloat16, kind="ExternalInput")
    out = nc.dram_tensor("out", [NS, OUT], mybir.dt.bfloat16, kind="ExternalOutput")

    with tile.TileContext(nc) as tc, ExitStack() as ctx:
        consts = ctx.enter_context(tc.tile_pool(name="consts", bufs=1))
        xp = ctx.enter_context(tc.tile_pool(name="xp", bufs=10))
        qp = ctx.enter_context(tc.tile_pool(name="qp", bufs=10))
        op = ctx.enter_context(tc.tile_pool(name="op", bufs=4))
        psum = ctx.enter_context(tc.tile_pool(name="psum", bufs=2, space="PSUM"))

        # ---- chunk 0 x-load + quantize first: the first matmul group only
        # needs x0 + wt[0], so x0 must not queue behind all 16 wt DMAs ----
        def load_chunk(c):
            # per-128-row tiles: contiguous DMA-transpose dst, and the first
            # matmul group only waits on its own 512KB slice
            n0 = c * N_CHUNK
            xqs = []
            for ns in range(N_CHUNK // 128):
                r0 = n0 + ns * 128
                xt = xp.tile([128, K_TILES, 128], mybir.dt.bfloat16, name="xt")
                nc.sync.dma_start_transpose(xt[:], x[r0:r0 + 128, :])
                xq = qp.tile([128, K_TILES, 128], mybir.dt.float8e4, name="xq")
                if scale != 1.0:
                    nc.scalar.activation(xq[:], xt[:],
                                         mybir.ActivationFunctionType.Copy,
                                         scale=1.0 / scale)
                else:
                    nc.scalar.copy(xq[:], xt[:])
                xqs.append(xq)
            return xqs

        xq0 = load_chunk(0)

        # ---- constants: W^T (DMA-transposed), broadcast bias ----
        # one tile per 128-wide k-chunk so matmuls start as chunks land
        wt_tiles = []
        for k in range(K_TILES):
            wtk = consts.tile([128, OUT], mybir.dt.bfloat16, name=f"wt{k}")
            nc.sync.dma_start_transpose(wtk[:], w[:, k * 128:(k + 1) * 128])
            if scale != 1.0:
                wsk = consts.tile([128, OUT], mybir.dt.bfloat16, name=f"ws{k}")
                nc.vector.tensor_scalar_mul(wsk[:], wtk[:], scale)
                wtk = wsk
            wt_tiles.append(wtk)

        bias_row = consts.tile([1, OUT], mybir.dt.bfloat16)
        nc.scalar.dma_start(bias_row[:], b.rearrange("(p o) -> p o", p=1))
        ones_col = consts.tile([1, 128], mybir.dt.bfloat16)
        nc.vector.memset(ones_col[:], 1.0)
        bias_bc = consts.tile([128, OUT], mybir.dt.float32)
        for ob in range(O_BANKS):
            pt = psum.tile([128, 512], mybir.dt.float32, name="pt", tag="acc0")
            nc.tensor.matmul(pt[:], ones_col[:], bias_row[:, ob * 512:(ob + 1) * 512])
            nc.scalar.copy(bias_bc[:, ob * 512:(ob + 1) * 512], pt[:])

        # ---- main loop ----
        for c in range(NS // N_CHUNK):
            n0 = c * N_CHUNK
            xqs = xq0 if c == 0 else load_chunk(c)

            for ns in range(N_CHUNK // 128):
                ot = op.tile([128, OUT], mybir.dt.bfloat16)
                pts = [psum.tile([128, 512], mybir.dt.float32, name=f"acc{ob}", tag=f"acc{ob}")
                       for ob in range(O_BANKS)]
                for k in range(K_TILES):
                    for ob in range(O_BANKS):
                        nc.tensor.matmul(
                            pts[ob][:],
                            xqs[ns][:, k, :],
                            wt_tiles[k][:, ob * 512:(ob + 1) * 512],
                            start=(k == 0), stop=(k == K_TILES - 1))
                for ob in range(O_BANKS):
                    nc.vector.tensor_add(
                        ot[:, ob * 512:(ob + 1) * 512], pts[ob][:],
                        bias_bc[:, ob * 512:(ob + 1) * 512])
                nc.scalar.dma_start(out[n0 + ns * 128:n0 + (ns + 1) * 128, :], ot[:])
    nc.finalize()
    return nc


def kernel(x, weight, bias, input_scale, _trace=False):
    s = float(np.asarray(input_scale).reshape(-1)[0])
    if s not in _cache:
        _cache[s] = build(s)
    nc = _cache[s]
    weight = np.ascontiguousarray(weight)
    bias = np.ascontiguousarray(bias)
    in_maps = [
        {"x": np.ascontiguousarray(x[i * NS:(i + 1) * NS]),
         "weight": weight, "bias": bias}
        for i in range(N_CORES)
    ]
    res = run_bass_kernel_spmd(nc, in_maps, core_ids=list(range(N_CORES)),
                               trace=_trace)
    outs = [res.results[i]["out"] for i in range(N_CORES)]
    full = np.concatenate(outs, axis=0)
    if _trace:
        return full, res
    return full
